# revision 24
# baseline (speedup 1.0000x reference)
"""Trainium2 Bass kernel for nn_CrossAttention_47502338294587.

Math: the reference cross-attention has a single KV position broadcast over
all T query positions.  Softmax over a row of identical logits is uniform,
so attention output == v for every query, and the whole module collapses to

    out[b, t, :] = (visual_features[b] @ Wv + bv) @ Wp + bp      (for all t)

independent of x / Wq / Wk.  The device computes the two projections and
broadcasts the per-batch row over the T axis; the host only does input
layout prep (incl. f32->bf16 weight re-encoding) and shard re-assembly.

Sharding: tensor-parallel over the output channel dim C — core i computes
and writes out[:, :, i*128:(i+1)*128].

Per-core structure (v4):
  * PE warm-up burst during the NEFF prelude + Wv DMA window so the HAM
    clock gate is at full rate when the real matmuls arrive.
  * vv is computed TRANSPOSED, chunk-pipelined behind the Wv stream:
      vvT_j[n, b] = sum_k Wv[k, j*128+n] vf[b, k]   (Wv chunks stationary)
    so vv^T materializes directly (no PE transposes) and mm2 accumulates
    per chunk right behind the DMA stream:
      row[b, c'] += vvT_j^T @ Wp[j-chunk, ci]
  * bv folded into the vvT PSUM->SBUF copy, bp into the row PSUM->SBUF
    copy (DVE adds with host-packed broadcast bias tiles).
  * T-broadcast: 4 tiny matmuls (one per b) with an all-ones [1,128]
    stationary and row_sb[b:b+1,:] as the 128-col moving operand write
    psum_bc[t, b-block] directly; then a split DVE/GpSimd copy to bf16
    and one replicated DMA (step-0 source over the 8 t-chunks) writes the
    whole bf16 output shard; host upcasts.
"""

import os
import sys

import numpy as np

for _p in ("/opt/trn_rl_repo",):
    if _p not in sys.path and os.path.isdir(_p):
        sys.path.insert(0, _p)

B, T, C = 4, 1024, 1024
N_CORES = 8
CSH = C // N_CORES  # 128, C-shard per core
KC = C // 128  # 8 chunks of the contraction / column-group dims
NWV = 2  # wv arrives in 2 DMA parts (4 column-groups each, 8KB descriptors)
N_WARM_BIG = 16  # PE warm-up matmuls, 512 cols each
N_WARM_SMALL = 4  # fine-grained warm-up tail, 128 cols each

_BUILT = None


def build_nc():
    """Build + compile the Bass program (one NeuronCore's SPMD body)."""
    import concourse.bass as bass
    import concourse.mybir as mybir
    import concourse.tile as tile
    from concourse import bacc

    f32 = mybir.dt.float32
    bf16 = mybir.dt.bfloat16
    nc = bacc.Bacc("TRN2", target_bir_lowering=False, debug=False)

    # host pre-packs everything into the exact SBUF layouts (layout prep):
    # wvj[p, j, k, n] = Wv[k*128+p, j*128+n]   (bf16)
    wvj = nc.dram_tensor("wvj", [128, KC * KC * 128], bf16, kind="ExternalInput")
    # wp[p, j, c] = Wp[j*128+p, ci_c]          (bf16)
    wp = nc.dram_tensor("wp", [128, KC * CSH], bf16, kind="ExternalInput")
    # vft[p, k, b] = vf[b, k*128+p]            (bf16)
    vft = nc.dram_tensor("vft", [128, KC * B], bf16, kind="ExternalInput")
    # bvt[p, j] = bv[j*128+p]                  (f32)
    bvt = nc.dram_tensor("bvt", [128, KC], f32, kind="ExternalInput")
    # bpc = bp[ci]                             (bf16)
    bpc = nc.dram_tensor("bpc", [1, CSH], bf16, kind="ExternalInput")
    # out[t, b*CSH + c]; host re-assembles full[b, t, ci] = out[t, b-block]
    out = nc.dram_tensor("out", [T, B * CSH], bf16, kind="ExternalOutput")

    def rep_ap(ap, n, axis=1):
        """insert a step-0 (replicating) dim of extent n at `axis`."""
        dims = [list(d) for d in ap.ap]
        dims.insert(axis, [0, n])
        return bass.AP(ap.tensor, ap.offset, dims)

    def band_select(ap, mult, width):
        """keep 1.0 inside the band 0 <= y - mult*k <= width-1, else 0."""
        nc.gpsimd.memset(ap, 1.0)
        nc.gpsimd.affine_select(
            out=ap, in_=ap, compare_op=mybir.AluOpType.is_ge, fill=0.0,
            base=0, pattern=[[1, ap.shape[-1]]], channel_multiplier=-mult,
        )
        nc.gpsimd.affine_select(
            out=ap, in_=ap, compare_op=mybir.AluOpType.is_ge, fill=0.0,
            base=width - 1, pattern=[[-1, ap.shape[-1]]], channel_multiplier=mult,
        )

    with tile.TileContext(nc) as tc:
        with tc.tile_pool(name="sb", bufs=1) as sb:
            # ---- SBUF tiles -------------------------------------------------
            wv_t = sb.tile([128, KC, KC, 128], bf16, tag="wv")
            wp_t = sb.tile([128, KC, CSH], bf16, tag="wp")
            vft_t = sb.tile([128, KC, B], bf16, tag="vft")
            bvt_t = sb.tile([128, KC], f32, tag="bvt")
            bpc_t = sb.tile([1, CSH], bf16, tag="bpc")
            ones1 = sb.tile([1, B], bf16, tag="ones1")
            ones4 = sb.tile([B, 128], bf16, tag="ones4")
            sel_t = sb.tile([B, B * CSH], f32, tag="sel")
            vvt_sb = sb.tile([128, KC, B], bf16, tag="vvt")
            rhs4_t = sb.tile([B, B * CSH], bf16, tag="rhs4")
            bc_t = sb.tile([128, B * CSH], bf16, tag="bc")
            dummy = sb.tile([128, 512], bf16, tag="dummy")

            # gpsimd exits the prelude first — let it arm the PE warm-up
            nc.gpsimd.memset(dummy[:], 0.5)
            nc.vector.memset(ones1[:], 1.0)
            nc.vector.memset(ones4[:], 1.0)
            band_select(sel_t[:], CSH, CSH)

            # ---- DMA in (pipeline-front first on each queue) ----------------
            # wv parts sized [4,3,1] j-chunks: the tiny last part minimizes
            # (last completion sem) + (PE work left after it)
            wv_src = wvj.rearrange("p (j k n) -> p j k n", k=KC, n=128)
            for js in (slice(0, 4), slice(4, 7), slice(7, 8)):
                nc.sync.dma_start(wv_t[:, js, :, :], wv_src[:, js, :, :])
            nc.scalar.dma_start(vft_t[:], vft.rearrange("p (k b) -> p k b", b=B))
            nc.scalar.dma_start(wp_t[:], wp.rearrange("p (j c) -> p j c", c=CSH))
            nc.scalar.dma_start(bvt_t[:], bvt.rearrange("p k -> p k"))
            nc.scalar.dma_start(bpc_t[:], bpc[:, :])

            with (
                tc.tile_pool(name="pw", bufs=1, space="PSUM") as pw,
                tc.tile_pool(name="pj", bufs=4, space="PSUM") as pj,
                tc.tile_pool(name="pr", bufs=1, space="PSUM") as pr,
                tc.tile_pool(name="pb", bufs=1, space="PSUM") as pb,
            ):
                # ---- PE warm-up (results discarded) -------------------------
                psum_w = pw.tile([128, 512], f32, tag="warm")
                for w in range(N_WARM_BIG):
                    nc.tensor.matmul(
                        psum_w[:], dummy[:, 0:128], dummy[:], start=True, stop=True
                    )
                for w in range(N_WARM_SMALL):
                    nc.tensor.matmul(
                        psum_w[:, 0:128],
                        dummy[:, 0:128],
                        dummy[:, 0:128],
                        start=True,
                        stop=True,
                    )

                # ---- chunk-pipelined mm1T (vvT_j) + mm2 accumulation --------
                psum_row = pr.tile([B, CSH], f32, tag="row")
                # bp[ci] first (K=1 rank-1 row) — keeps it off the tail
                nc.tensor.matmul(
                    psum_row[:], ones1[0:1, :], bpc_t[0:1, :], start=True, stop=False
                )
                for j in range(KC):
                    # vvT_j[n, b] = sum_k Wv[k, jn]^T vfT[k, b]
                    psum_j = pj.tile([128, B], f32, tag="vvtj")
                    for k in range(KC):
                        nc.tensor.matmul(
                            psum_j[:],
                            wv_t[:, j, k, :],
                            vft_t[:, k, :],
                            start=(k == 0),
                            stop=(k == KC - 1),
                        )
                    # software pipeline: issue mm2_{j-1} after mm1T_j so the
                    # PE never stalls on the DVE copy of the current chunk
                    if j > 0:
                        nc.tensor.matmul(
                            psum_row[:],
                            vvt_sb[:, j - 1, :],
                            wp_t[:, j - 1, :],
                            start=False,
                            stop=False,
                        )
                    # vvT_j + bv_j (b-replicated), cast to bf16
                    nc.vector.tensor_add(
                        vvt_sb[:, j, :], psum_j[:], rep_ap(bvt_t[:, j : j + 1], B)
                    )
                nc.tensor.matmul(
                    psum_row[:],
                    vvt_sb[:, KC - 1, :],
                    wp_t[:, KC - 1, :],
                    start=False,
                    stop=True,
                )

                # ---- broadcast: rhs4 = rep4(row) * sel, bc = ones^T @ rhs4 --
                # pipelined in two (b0,b1)/(b2,b3) halves: mul -> bcast mm ->
                # bf16 copy -> replicated out-DMA (sync / scalar queues)
                half = B * CSH // 2
                rhs4_v = rhs4_t[:].rearrange("p (q f) -> p q f", q=B)
                sel_v = sel_t[:].rearrange("p (q f) -> p q f", q=B)
                psum_bc = pb.tile([128, B * CSH], f32, tag="bc")
                out_v = out.rearrange("(q p) f -> p q f", p=128)
                for h in range(2):
                    qs = slice(2 * h, 2 * h + 2)
                    fs = slice(h * half, (h + 1) * half)
                    nc.vector.tensor_mul(
                        rhs4_v[:, qs, :],
                        rep_ap(psum_row[:], 2),
                        sel_v[:, qs, :],
                    )
                    nc.tensor.matmul(
                        psum_bc[:, fs], ones4[:], rhs4_t[:, fs], start=True, stop=True
                    )
                    nc.vector.tensor_copy(bc_t[:, fs], psum_bc[:, fs])
                # single full-width replicated DMA keeps 1KB descriptors
                nc.sync.dma_start(out_v[:, :, :], rep_ap(bc_t[:], KC))

    nc.compile()
    return nc


def _get_built():
    global _BUILT
    if _BUILT is None:
        _BUILT = build_nc()
    return _BUILT


def make_in_maps(inputs):
    import ml_dtypes

    bf = ml_dtypes.bfloat16
    vf = np.asarray(inputs["visual_features"], np.float32)
    wv = np.asarray(inputs["Wv"], np.float32)
    wp = np.asarray(inputs["Wp"], np.float32)
    bv = np.asarray(inputs["bv"], np.float32)
    bp = np.asarray(inputs["bp"], np.float32)
    # wvj[p, j*KC*128 + k*128 + n] = Wv[k*128+p, j*128+n]
    wvj = np.ascontiguousarray(
        wv.reshape(KC, 128, KC, 128).transpose(1, 2, 0, 3).reshape(128, KC * KC * 128)
    ).astype(bf)
    # vft[p, k*B + b] = vf[b, k*128+p]
    vft = np.ascontiguousarray(
        vf.T.reshape(KC, 128, B).transpose(1, 0, 2).reshape(128, KC * B)
    ).astype(bf)
    # bvt[p, j] = bv[j*128+p]
    bvt = np.ascontiguousarray(bv.reshape(KC, 128).T)
    maps = []
    for i in range(N_CORES):
        ci = slice(i * CSH, (i + 1) * CSH)
        # wp_p[p, j*CSH + c] = Wp[j*128+p, ci_c]
        wp_p = np.ascontiguousarray(
            wp[:, ci].reshape(KC, 128, CSH).transpose(1, 0, 2).reshape(128, KC * CSH)
        ).astype(bf)
        bpc = bp[ci].reshape(1, CSH).astype(bf)
        maps.append({"wvj": wvj, "wp": wp_p, "vft": vft, "bvt": bvt, "bpc": bpc})
    return maps


def run(inputs, trace=False, **kw):
    from concourse.bass_utils import run_bass_kernel_spmd

    nc = _get_built()
    res = run_bass_kernel_spmd(
        nc,
        make_in_maps(inputs),
        core_ids=list(range(N_CORES)),
        trace=trace,
        **kw,
    )
    full = np.empty((B, T, C), np.float32)
    for i, r in enumerate(res.results):
        shard = np.asarray(r["out"]).astype(np.float32)  # [T, B*CSH]
        full[:, :, i * CSH : (i + 1) * CSH] = shard.reshape(T, B, CSH).transpose(
            1, 0, 2
        )
    return full, res


def kernel(**inputs) -> np.ndarray:
    full, _ = run(inputs, trace=False)
    return full


# revision 25
# speedup vs baseline: 1.1183x; 1.1183x over previous
"""Trainium2 Bass kernel for nn_CrossAttention_47502338294587.

Math: the reference cross-attention has a single KV position broadcast over
all T query positions.  Softmax over a row of identical logits is uniform,
so attention output == v for every query, and the whole module collapses to

    out[b, t, :] = (visual_features[b] @ Wv + bv) @ Wp + bp      (for all t)

independent of x / Wq / Wk.  The device computes the two projections and
broadcasts the per-batch row over the T axis; the host only does input
layout prep (incl. f32->bf16 weight re-encoding) and shard re-assembly.

Sharding: tensor-parallel over the output channel dim C — core i computes
and writes out[:, :, i*128:(i+1)*128].

Per-core structure (v4):
  * PE warm-up burst during the NEFF prelude + Wv DMA window so the HAM
    clock gate is at full rate when the real matmuls arrive.
  * vv is computed TRANSPOSED, chunk-pipelined behind the Wv stream:
      vvT_j[n, b] = sum_k Wv[k, j*128+n] vf[b, k]   (Wv chunks stationary)
    so vv^T materializes directly (no PE transposes) and mm2 accumulates
    per chunk right behind the DMA stream:
      row[b, c'] += vvT_j^T @ Wp[j-chunk, ci]
  * bv folded into the vvT PSUM->SBUF copy, bp into the row PSUM->SBUF
    copy (DVE adds with host-packed broadcast bias tiles).
  * T-broadcast: 4 tiny matmuls (one per b) with an all-ones [1,128]
    stationary and row_sb[b:b+1,:] as the 128-col moving operand write
    psum_bc[t, b-block] directly; then a split DVE/GpSimd copy to bf16
    and one replicated DMA (step-0 source over the 8 t-chunks) writes the
    whole bf16 output shard; host upcasts.
"""

import os
import sys

import numpy as np

for _p in ("/opt/trn_rl_repo",):
    if _p not in sys.path and os.path.isdir(_p):
        sys.path.insert(0, _p)

B, T, C = 4, 1024, 1024
N_CORES = 8
CSH = C // N_CORES  # 128, C-shard per core
KC = C // 128  # 8 chunks of the contraction / column-group dims
NWV = 2  # wv arrives in 2 DMA parts (4 column-groups each, 8KB descriptors)
N_WARM_BIG = 16  # PE warm-up matmuls, 512 cols each
N_WARM_SMALL = 4  # fine-grained warm-up tail, 128 cols each

_BUILT = None


def build_nc():
    """Build + compile the Bass program (one NeuronCore's SPMD body)."""
    import concourse.bass as bass
    import concourse.mybir as mybir
    import concourse.tile as tile
    from concourse import bacc

    f32 = mybir.dt.float32
    bf16 = mybir.dt.bfloat16
    nc = bacc.Bacc("TRN2", target_bir_lowering=False, debug=False)

    # host pre-packs everything into the exact SBUF layouts (layout prep):
    # wvj[p, j, k, n] = Wv[k*128+p, j*128+n]   (bf16)
    wvj = nc.dram_tensor("wvj", [128, KC * KC * 128], bf16, kind="ExternalInput")
    # wp[p, j, c] = Wp[j*128+p, ci_c]          (bf16)
    wp = nc.dram_tensor("wp", [128, KC * CSH], bf16, kind="ExternalInput")
    # vft[p, k, b] = vf[b, k*128+p]            (bf16)
    vft = nc.dram_tensor("vft", [128, KC * B], bf16, kind="ExternalInput")
    # bvt[p, j] = bv[j*128+p]                  (f32)
    bvt = nc.dram_tensor("bvt", [128, KC], f32, kind="ExternalInput")
    # bpc = bp[ci]                             (bf16)
    bpc = nc.dram_tensor("bpc", [1, CSH], bf16, kind="ExternalInput")
    # out[t, b*CSH + c]; host re-assembles full[b, t, ci] = out[t, b-block]
    out = nc.dram_tensor("out", [T, B * CSH], bf16, kind="ExternalOutput")

    def rep_ap(ap, n, axis=1):
        """insert a step-0 (replicating) dim of extent n at `axis`."""
        dims = [list(d) for d in ap.ap]
        dims.insert(axis, [0, n])
        return bass.AP(ap.tensor, ap.offset, dims)

    def band_select(ap, mult, width):
        """keep 1.0 inside the band 0 <= y - mult*k <= width-1, else 0."""
        nc.gpsimd.memset(ap, 1.0)
        nc.gpsimd.affine_select(
            out=ap, in_=ap, compare_op=mybir.AluOpType.is_ge, fill=0.0,
            base=0, pattern=[[1, ap.shape[-1]]], channel_multiplier=-mult,
        )
        nc.gpsimd.affine_select(
            out=ap, in_=ap, compare_op=mybir.AluOpType.is_ge, fill=0.0,
            base=width - 1, pattern=[[-1, ap.shape[-1]]], channel_multiplier=mult,
        )

    with tile.TileContext(nc) as tc:
        with tc.tile_pool(name="sb", bufs=1) as sb:
            # ---- SBUF tiles -------------------------------------------------
            wv_t = sb.tile([128, KC, KC, 128], bf16, tag="wv")
            wp_t = sb.tile([128, KC, CSH], bf16, tag="wp")
            vft_t = sb.tile([128, KC, B], bf16, tag="vft")
            bvt_t = sb.tile([128, KC], f32, tag="bvt")
            bpc_t = sb.tile([1, CSH], bf16, tag="bpc")
            ones1 = sb.tile([1, B], bf16, tag="ones1")
            ones4 = sb.tile([B, 128], bf16, tag="ones4")
            sel_t = sb.tile([B, B * CSH], f32, tag="sel")
            vvt_sb = sb.tile([128, KC, B], bf16, tag="vvt")
            rhs4_t = sb.tile([B, B * CSH], bf16, tag="rhs4")
            bc_t = sb.tile([128, B * CSH], bf16, tag="bc")
            dummy = sb.tile([128, 512], bf16, tag="dummy")

            # gpsimd exits the prelude first — let it arm the PE warm-up
            nc.gpsimd.memset(dummy[:], 0.5)
            nc.vector.memset(ones1[:], 1.0)
            nc.vector.memset(ones4[:], 1.0)
            band_select(sel_t[:], CSH, CSH)

            # ---- DMA in (pipeline-front first on each queue) ----------------
            # wv parts sized [4,3,1] j-chunks: the tiny last part minimizes
            # (last completion sem) + (PE work left after it)
            wv_src = wvj.rearrange("p (j k n) -> p j k n", k=KC, n=128)
            for js in (slice(0, 4), slice(4, 8)):
                nc.sync.dma_start(wv_t[:, js, :, :], wv_src[:, js, :, :])
            nc.scalar.dma_start(vft_t[:], vft.rearrange("p (k b) -> p k b", b=B))
            nc.scalar.dma_start(bpc_t[:], bpc[:, :])
            nc.scalar.dma_start(bvt_t[:], bvt.rearrange("p k -> p k"))
            nc.scalar.dma_start(wp_t[:], wp.rearrange("p (j c) -> p j c", c=CSH))

            with (
                tc.tile_pool(name="pw", bufs=1, space="PSUM") as pw,
                tc.tile_pool(name="pj", bufs=4, space="PSUM") as pj,
                tc.tile_pool(name="pr", bufs=1, space="PSUM") as pr,
                tc.tile_pool(name="pb", bufs=1, space="PSUM") as pb,
            ):
                # ---- PE warm-up (results discarded) -------------------------
                psum_w = pw.tile([128, 512], f32, tag="warm")
                for w in range(N_WARM_BIG):
                    nc.tensor.matmul(
                        psum_w[:], dummy[:, 0:128], dummy[:], start=True, stop=True
                    )
                for w in range(N_WARM_SMALL):
                    nc.tensor.matmul(
                        psum_w[:, 0:128],
                        dummy[:, 0:128],
                        dummy[:, 0:128],
                        start=True,
                        stop=True,
                    )

                # ---- chunk-pipelined mm1T (vvT_j) + mm2 accumulation --------
                psum_row = pr.tile([B, CSH], f32, tag="row")
                # bp[ci] first (K=1 rank-1 row) — keeps it off the tail
                nc.tensor.matmul(
                    psum_row[:], ones1[0:1, :], bpc_t[0:1, :], start=True, stop=False
                )
                for j in range(KC):
                    # vvT_j[n, b] = sum_k Wv[k, jn]^T vfT[k, b]
                    psum_j = pj.tile([128, B], f32, tag="vvtj")
                    for k in range(KC):
                        nc.tensor.matmul(
                            psum_j[:],
                            wv_t[:, j, k, :],
                            vft_t[:, k, :],
                            start=(k == 0),
                            stop=(k == KC - 1),
                        )
                    # software pipeline: issue mm2_{j-1} after mm1T_j so the
                    # PE never stalls on the DVE copy of the current chunk
                    if j > 0:
                        nc.tensor.matmul(
                            psum_row[:],
                            vvt_sb[:, j - 1, :],
                            wp_t[:, j - 1, :],
                            start=False,
                            stop=False,
                        )
                    # vvT_j + bv_j (b-replicated), cast to bf16
                    nc.vector.tensor_add(
                        vvt_sb[:, j, :], psum_j[:], rep_ap(bvt_t[:, j : j + 1], B)
                    )
                nc.tensor.matmul(
                    psum_row[:],
                    vvt_sb[:, KC - 1, :],
                    wp_t[:, KC - 1, :],
                    start=False,
                    stop=True,
                )

                # ---- broadcast: rhs4 = rep4(row) * sel, bc = ones^T @ rhs4 --
                # pipelined in two (b0,b1)/(b2,b3) halves: mul -> bcast mm ->
                # bf16 copy -> replicated out-DMA (sync / scalar queues)
                half = B * CSH // 2
                rhs4_v = rhs4_t[:].rearrange("p (q f) -> p q f", q=B)
                sel_v = sel_t[:].rearrange("p (q f) -> p q f", q=B)
                psum_bc = pb.tile([128, B * CSH], f32, tag="bc")
                out_v = out.rearrange("(q p) f -> p q f", p=128)
                for h in range(2):
                    qs = slice(2 * h, 2 * h + 2)
                    fs = slice(h * half, (h + 1) * half)
                    nc.vector.tensor_mul(
                        rhs4_v[:, qs, :],
                        rep_ap(psum_row[:], 2),
                        sel_v[:, qs, :],
                    )
                    nc.tensor.matmul(
                        psum_bc[:, fs], ones4[:], rhs4_t[:, fs], start=True, stop=True
                    )
                    nc.vector.tensor_copy(bc_t[:, fs], psum_bc[:, fs])
                # single full-width replicated DMA keeps 1KB descriptors
                nc.sync.dma_start(out_v[:, :, :], rep_ap(bc_t[:], KC))

    nc.compile()
    return nc


def _get_built():
    global _BUILT
    if _BUILT is None:
        _BUILT = build_nc()
    return _BUILT


def make_in_maps(inputs):
    import ml_dtypes

    bf = ml_dtypes.bfloat16
    vf = np.asarray(inputs["visual_features"], np.float32)
    wv = np.asarray(inputs["Wv"], np.float32)
    wp = np.asarray(inputs["Wp"], np.float32)
    bv = np.asarray(inputs["bv"], np.float32)
    bp = np.asarray(inputs["bp"], np.float32)
    # wvj[p, j*KC*128 + k*128 + n] = Wv[k*128+p, j*128+n]
    wvj = np.ascontiguousarray(
        wv.reshape(KC, 128, KC, 128).transpose(1, 2, 0, 3).reshape(128, KC * KC * 128)
    ).astype(bf)
    # vft[p, k*B + b] = vf[b, k*128+p]
    vft = np.ascontiguousarray(
        vf.T.reshape(KC, 128, B).transpose(1, 0, 2).reshape(128, KC * B)
    ).astype(bf)
    # bvt[p, j] = bv[j*128+p]
    bvt = np.ascontiguousarray(bv.reshape(KC, 128).T)
    maps = []
    for i in range(N_CORES):
        ci = slice(i * CSH, (i + 1) * CSH)
        # wp_p[p, j*CSH + c] = Wp[j*128+p, ci_c]
        wp_p = np.ascontiguousarray(
            wp[:, ci].reshape(KC, 128, CSH).transpose(1, 0, 2).reshape(128, KC * CSH)
        ).astype(bf)
        bpc = bp[ci].reshape(1, CSH).astype(bf)
        maps.append({"wvj": wvj, "wp": wp_p, "vft": vft, "bvt": bvt, "bpc": bpc})
    return maps


def run(inputs, trace=False, **kw):
    from concourse.bass_utils import run_bass_kernel_spmd

    nc = _get_built()
    res = run_bass_kernel_spmd(
        nc,
        make_in_maps(inputs),
        core_ids=list(range(N_CORES)),
        trace=trace,
        **kw,
    )
    full = np.empty((B, T, C), np.float32)
    for i, r in enumerate(res.results):
        shard = np.asarray(r["out"]).astype(np.float32)  # [T, B*CSH]
        full[:, :, i * CSH : (i + 1) * CSH] = shard.reshape(T, B, CSH).transpose(
            1, 0, 2
        )
    return full, res


def kernel(**inputs) -> np.ndarray:
    full, _ = run(inputs, trace=False)
    return full
